# revision 1
# baseline (speedup 1.0000x reference)
"""Trainium2 Bass kernel for the ANFIS forward pass (8-core data-parallel).

Math: with L[b,f,m] = -0.5*((X[b,f]-mu[f,m])/sigma[f,m])^2,
  miAlloc[b,r] = prod_f exp(L[b,f,rules[r,f]])
  out[b] = (miAlloc @ c) / (sum_r miAlloc + 1e-10),  c = consequents.sum(1)

Factor the 8 features into two halves of 4. Each half has 81 possible
membership tuples, so miAlloc[b,r] = W1[b,rho1(r)] * W2[b,rho2(r)] where
  W1[b,t] = exp(sum_{f<4} a[f,tf]*(X[b,f]-mu[f,tf])^2),  a = -0.5/sigma^2
and rho1/rho2 map each rule to its half-tuple index. Then with
  C2[t1,t2] = sum_{r: rho(r)=(t1,t2)} c[r],   D2[t1,t2] = #{r: rho(r)=(t1,t2)}
(exact for arbitrary `rules`, duplicates included):
  num[b] = sum_{t2} (C2^T W1T)[t2,b] * W2T[t2,b]
  den[b] = sum_{t2} (D2^T W1T)[t2,b] * W2T[t2,b]
  out[b] = num[b] / (den[b] + 1e-10)

Per core (batch shard of 1024): one Square activation, two K=12 bf16
matmuls, exp over [81,1024], two K=81 bf16 matmuls, elementwise product,
ones-reduce matmul, then 1/(den+eps) via exp(-ln(den+eps)) on ScalarE.
"""

import numpy as np
import ml_dtypes

import concourse.bass as bass
import concourse.tile as tile
from concourse import bacc, mybir
from concourse.bass_utils import run_bass_kernel_spmd

B, F, M = 8192, 8, 3
NC = 8
BC = B // NC  # 1024 batch rows per core
T = M**4  # 81 tuples per feature-half
FP32 = mybir.dt.float32
BF16 = mybir.dt.bfloat16
AF = mybir.ActivationFunctionType
NP_BF16 = ml_dtypes.bfloat16

_CACHE = {}


def _build_graph():
    nc = bacc.Bacc("TRN2", target_bir_lowering=False, debug=False, num_devices=NC)

    # xt: col 0 = -mu (per-partition bias), cols 1.. = X^T replicated rows
    xt_ext = nc.dram_tensor("xt", [44, 1 + BC], BF16, kind="ExternalInput").ap()
    # bigc: [81, 275] bf16 = C2 | D2 | ones32 | eb (eb on rows 0:44)
    bigc_ext = nc.dram_tensor("bigc", [T, 3 * T + 32], BF16, kind="ExternalInput").ap()
    out_ext = nc.dram_tensor("out", [1, BC], FP32, kind="ExternalOutput").ap()

    with tile.TileContext(nc) as tc:
        with (
            tc.tile_pool(name="const", bufs=1) as const,
            tc.tile_pool(name="work", bufs=1) as work,
            tc.tile_pool(name="psum", bufs=1, space=bass.MemorySpace.PSUM) as psum,
        ):
            # column-split DMAs on the Sync HWDGE queue (Scalar's queue is
            # busy with the ACT table load): the batch-half pipeline can
            # start as soon as the first half's columns land
            HB = BC // 2  # 512-column half

            # first-half columns as one sync DMA (HWDGE, gates SQUARE h0);
            # second half split across scalar (free once its ACT table load
            # finishes) and sync; constants on gpsimd (needed only by the
            # later matmuls, so SWDGE latency is hidden)
            xt = const.tile([44, 1 + BC], BF16)
            nc.sync.dma_start(out=xt[:, 0 : 1 + HB], in_=xt_ext[:, 0 : 1 + HB])
            nc.scalar.dma_start(
                out=xt[0:22, 1 + HB :], in_=xt_ext[0:22, 1 + HB :]
            )
            nc.sync.dma_start(out=xt[22:44, 1 + HB :], in_=xt_ext[22:44, 1 + HB :])
            bigc = const.tile([T, 3 * T + 32], BF16)
            nc.gpsimd.dma_start(out=bigc[:, :], in_=bigc_ext[:, :])
            c2 = bigc[:, 0:T]
            d2 = bigc[:, T : 2 * T]
            ones32 = bigc[:, 2 * T : 2 * T + 32]
            eb_q0 = bigc[0:12, 2 * T + 32 : 3 * T + 32]
            eb_q32 = bigc[32:44, 2 * T + 32 : 3 * T + 32]

            # Per batch-half pipeline, den-path first (it gates the epilogue):
            #   sq -> logW matmuls -> exp(W1h) -> exp(W2h)
            #   hd -> pd=hd*w2 -> den-reduce ; ht -> p1=ht*w2 -> num-reduce
            # logW quarters get separate PSUM tiles so each exp waits only on
            # its own matmul
            sq = work.tile([44, BC], BF16)
            lw1h = [
                psum.tile([T, HB], FP32, tag=f"pa{h}", name=f"lw1h{h}")
                for h in range(2)
            ]
            lw2h = [
                psum.tile([T, HB], FP32, tag=f"pb{h}", name=f"lw2h{h}")
                for h in range(2)
            ]
            w = work.tile([T, 2 * BC], BF16)
            w1 = w[:, 0:BC]
            w2 = w[:, BC : 2 * BC]
            ht = psum.tile([T, BC], FP32, tag="pc")
            hd = psum.tile([T, BC], FP32, tag="pd")
            p1 = work.tile([T, BC], BF16)
            pd = work.tile([T, BC], BF16)
            # each half's reduction is broadcast onto a 32-row block
            # (rows 0:32 / 32:64) so the epilogue reads only initialized
            # partitions, on two lanes (rows 0 and 32 are DMA'd out)
            den = psum.tile([64, HB], FP32, tag="pa0")
            num = psum.tile([64, HB], FP32, tag="pa1")

            for h in range(2):
                s = bass.ts(h, HB)
                # rows 12:32 of sq are never read downstream; garbage is fine
                nc.scalar.activation(
                    sq[:, s],
                    xt[:, bass.ds(1 + h * HB, HB)],
                    AF.Square,
                    bias=xt[:, 0:1],
                )
                nc.tensor.matmul(lw1h[h][:, :], lhsT=eb_q0, rhs=sq[0:12, s])
                nc.tensor.matmul(lw2h[h][:, :], lhsT=eb_q32, rhs=sq[32:44, s])
            # interleave the exps (w1h, w2h per half) so the h0 product chain
            # unblocks after two ACT ops instead of five
            for h in range(2):
                nc.scalar.activation(w[:, bass.ts(h, HB)], lw1h[h][:, :], AF.Exp)
                nc.scalar.activation(
                    w[:, bass.ds(BC + h * HB, HB)], lw2h[h][:, :], AF.Exp
                )
            for h in range(2):
                s = bass.ts(h, HB)
                nc.tensor.matmul(hd[:, s], lhsT=d2, rhs=w1[:, s])
                nc.tensor.matmul(ht[:, s], lhsT=c2, rhs=w1[:, s])
            for h in range(2):
                s = bass.ts(h, HB)
                po = bass.ds(32 * h, 32)
                nc.vector.tensor_mul(pd[:, s], hd[:, s], w2[:, s])
                nc.tensor.matmul(den[po, :], lhsT=ones32, rhs=pd[:, s])
                nc.vector.tensor_mul(p1[:, s], ht[:, s], w2[:, s])
                nc.tensor.matmul(num[po, :], lhsT=ones32, rhs=p1[:, s])

            # out = num / den; den >= W(best rule) >> 1e-10 for any plausible
            # input, so the reference's +1e-10 is numerically invisible and
            # dropped; approx recip is ~51 ULP, plenty for the 2e-2 gate
            rden = work.tile([64, HB], FP32)
            nc.vector.reciprocal_approx_fast(rden[:, :], den[:, :])
            outt = work.tile([64, HB], FP32)
            nc.vector.tensor_mul(outt[:, :], num[:, :], rden[:, :])

            nc.sync.dma_start(out=out_ext[:, 0:HB], in_=outt[0:1, :])
            nc.scalar.dma_start(out=out_ext[:, HB:BC], in_=outt[32:33, :])

    nc.compile()
    return nc


def _get_graph():
    if "nc" not in _CACHE:
        _CACHE["nc"] = _build_graph()
    return _CACHE["nc"]


def _prep_inputs(X, mu, sigma, consequents, rules):
    X = np.ascontiguousarray(np.asarray(X, dtype=np.float32))
    mu = np.asarray(mu, dtype=np.float32)
    sigma = np.asarray(sigma, dtype=np.float32)
    c = np.asarray(consequents, dtype=np.float32).sum(axis=1)
    r = np.asarray(rules).astype(np.int64)

    a = (-0.5 / (np.asarray(sigma, np.float64) ** 2)).astype(np.float32)  # [F, M]

    # tuple digit f of t (m0 major), t in [0, 81)
    digits = (np.arange(T)[:, None] // np.array([27, 9, 3, 1])[None, :]) % 3  # [81, 4]

    eb = np.zeros((44, T), np.float32)
    negmu = np.zeros((44, 1), np.float32)
    for f in range(4):
        for m in range(3):
            sel = (digits[:, f] == m).astype(np.float32)
            eb[3 * f + m, :] = a[f, m] * sel
            eb[32 + 3 * f + m, :] = a[4 + f, m] * sel
            negmu[3 * f + m, 0] = -mu[f, m]
            negmu[32 + 3 * f + m, 0] = -mu[4 + f, m]

    Xsh = X.reshape(NC, BC, F)
    xt = np.zeros((NC, 44, 1 + BC), np.float32)
    xt[:, :, 0] = negmu[None, :, 0]
    for f in range(4):
        for m in range(3):
            xt[:, 3 * f + m, 1:] = Xsh[:, :, f]
            xt[:, 32 + 3 * f + m, 1:] = Xsh[:, :, 4 + f]

    rho1 = ((r[:, 0] * 3 + r[:, 1]) * 3 + r[:, 2]) * 3 + r[:, 3]
    rho2 = ((r[:, 4] * 3 + r[:, 5]) * 3 + r[:, 6]) * 3 + r[:, 7]
    C2 = np.zeros((T, T), np.float64)
    np.add.at(C2, (rho1, rho2), c.astype(np.float64))
    D2 = np.zeros((T, T), np.float64)
    np.add.at(D2, (rho1, rho2), 1.0)

    bigc = np.zeros((T, 3 * T + 32), np.float32)
    bigc[:, 0:T] = C2.astype(np.float32)
    bigc[:, T : 2 * T] = D2.astype(np.float32)
    bigc[:, 2 * T : 2 * T + 32] = 1.0
    bigc[0:44, 2 * T + 32 :] = eb
    bigc = np.ascontiguousarray(bigc.astype(NP_BF16))

    in_maps = [
        {"xt": np.ascontiguousarray(xt[i].astype(NP_BF16)), "bigc": bigc}
        for i in range(NC)
    ]
    return in_maps


def _run(in_maps, trace=False, **kwargs):
    nc = _get_graph()
    return run_bass_kernel_spmd(
        nc, in_maps, core_ids=list(range(NC)), trace=trace, **kwargs
    )


def kernel(X, mu, sigma, consequents, rules):
    in_maps = _prep_inputs(X, mu, sigma, consequents, rules)
    res = _run(in_maps)
    out = np.concatenate(
        [np.asarray(res.results[i]["out"]).reshape(BC) for i in range(NC)]
    )
    return out.astype(np.float32)



# revision 7
# speedup vs baseline: 1.0723x; 1.0723x over previous
"""Trainium2 Bass kernel for the ANFIS forward pass (8-core data-parallel).

Math: with L[b,f,m] = -0.5*((X[b,f]-mu[f,m])/sigma[f,m])^2,
  miAlloc[b,r] = prod_f exp(L[b,f,rules[r,f]])
  out[b] = (miAlloc @ c) / (sum_r miAlloc + 1e-10),  c = consequents.sum(1)

Factor the 8 features into two halves of 4. Each half has 81 possible
membership tuples, so miAlloc[b,r] = W1[b,rho1(r)] * W2[b,rho2(r)] where
  W1[b,t] = exp(sum_{f<4} a[f,tf]*(X[b,f]-mu[f,tf])^2),  a = -0.5/sigma^2
and rho1/rho2 map each rule to its half-tuple index. With
  C2[t1,t2] = sum_{r: rho(r)=(t1,t2)} c[r],   D2[t1,t2] = #{r: rho(r)=(t1,t2)}
(exact for arbitrary `rules`, duplicates included):
  num[b] = sum_{t2} (C2^T W1)[t2,b] * W2[t2,b]
  den[b] = sum_{t2} (D2^T W1)[t2,b] * W2[t2,b]
  out[b] = num[b] / (den[b] + 1e-10)     <- divide happens on HOST

Device-side design notes (all fp16 data path, fp32 PSUM accumulation):
 * logW is computed as a single K=18 matmul over z = [x(8) | x^2(8) | 1 | 1]:
   a*(x-mu)^2 = a*x^2 - 2*a*mu*x + a*mu^2. The quadratic x^2 rows are squared
   in place on VectorE (2-byte 2x mode); the two ones-rows carry the constant
   term split hi/lo across two fp16 rows so it lands with ~fp32 precision.
 * exp() values are scaled by e^SHIFT per half to stay out of fp16 subnormals;
   the scale cancels in num/den (host divides with a rescaled epsilon).
 * The PE p-state ramps 0.65->1.2->2.4 GHz with ~3us of continuous work, so
   a run of warm-up matmuls on garbage SBUF keeps the array busy through the
   framework preamble + input DMA flight; real matmuls then run at full rate.
 * num/den are reduced by ones-matmuls into one [64,512] PSUM tile per half,
   copied to SBUF on ScalarE (DMA cannot read PSUM), and DMA'd out as two
   rows; the final divide is elementwise host post-processing of the gather.
"""

import numpy as np

import concourse.bass as bass
import concourse.tile as tile
from concourse import bacc, mybir
from concourse.bass_utils import run_bass_kernel_spmd

B, F, M = 8192, 8, 3
NC = 8
BC = B // NC  # 1024 batch rows per core
HB = BC // 2  # 512-column half
T = M**4  # 81 tuples per feature-half
K = 18  # x(8) | x^2(8) | ones(2)
FP32 = mybir.dt.float32
FP16 = mybir.dt.float16
AF = mybir.ActivationFunctionType
SHIFT = 2.0  # per-half exp scale; cancels in num/den
N_WARM = 14  # PE p-state warm-up matmuls

_CACHE = {}


def _build_graph():
    nc = bacc.Bacc("TRN2", target_bir_lowering=False, debug=False, num_devices=NC)

    xq_ext = nc.dram_tensor("xq", [K, BC], FP16, kind="ExternalInput").ap()
    # wb: A1 | A2 stage-1 weights, [18, 162]
    wb_ext = nc.dram_tensor("wb", [K, 2 * T], FP16, kind="ExternalInput").ap()
    # bigc: C2 | D2 | ones32, [81, 194]
    bigc_ext = nc.dram_tensor("bigc", [T, 2 * T + 32], FP16, kind="ExternalInput").ap()
    # o rows: num_h0, den_h0, num_h1, den_h1
    out_ext = nc.dram_tensor("o", [4, HB], FP32, kind="ExternalOutput").ap()

    with tile.TileContext(nc) as tc:
        with (
            tc.tile_pool(name="const", bufs=1) as const,
            tc.tile_pool(name="work", bufs=1) as work,
            tc.tile_pool(name="psum", bufs=1, space=bass.MemorySpace.PSUM) as psum,
        ):
            xq = const.tile([K, BC], FP16)
            wb = const.tile([K, 2 * T], FP16)
            bigc = const.tile([T, 2 * T + 32], FP16)
            c2 = bigc[:, 0:T]
            d2 = bigc[:, T : 2 * T]
            ones32 = bigc[:, 2 * T : 2 * T + 32]

            # input DMAs: batch-half h0 first (gates everything), then the
            # stage-1 weights, then h1; C2/D2/ones ride the gpsimd SWDGE queue
            # (needed ~2.5us after trigger, latency hidden)
            nc.sync.dma_start(out=xq[:, 0:HB], in_=xq_ext[:, 0:HB])
            nc.sync.dma_start(out=wb[:, :], in_=wb_ext[:, :])
            nc.sync.dma_start(out=xq[:, HB:BC], in_=xq_ext[:, HB:BC])
            nc.gpsimd.dma_start(out=bigc[:, :], in_=bigc_ext[:, :])

            # PSUM: 8 banks, tags reused once the lw tiles are consumed
            warm = psum.tile([T, HB], FP32, tag="pc", name="warm")
            lw = [
                psum.tile([T, HB], FP32, tag=t, name=f"lw{t}")
                for t in ("pa", "pb", "pc", "pd")
            ]  # w1h0, w2h0, w1h1, w2h1
            ht = [psum.tile([T, HB], FP32, tag=t, name=f"ht{t}") for t in ("pe", "pf")]
            hd = [psum.tile([T, HB], FP32, tag=t, name=f"hd{t}") for t in ("pg", "ph")]
            nd = [
                psum.tile([64, HB], FP32, tag=t, name=f"nd{t}") for t in ("pa", "pb")
            ]  # rows 0:32 num (row 0 used), 32:64 den (row 32 used)

            w = work.tile([T, 2 * BC], FP16)  # w1 cols 0:BC, w2 cols BC:2BC
            p = work.tile([T, 2 * BC], FP16)  # p1h0 | pdh0 | p1h1 | pdh1
            outt = [work.tile([33, HB], FP32, name=f"outt{h}") for h in range(2)]
            warm_l = work.tile([K, T], FP16)

            # PE warm-up: gated only on a tiny vector memset, so it runs from
            # the branch into the kernel body, ramping the p-state while the
            # input DMAs are in flight
            nc.vector.memset(warm_l[:, :], 0.0)
            for _ in range(N_WARM):
                nc.tensor.matmul(warm[:, 0:T], lhsT=warm_l[:, :], rhs=warm_l[:, :])

            # x^2 rows 0:8 squared in place (fp16 all-SBUF -> DVE 2x mode;
            # rows start at partition 0 to satisfy DVE partition alignment)
            for h in range(2):
                s = bass.ts(h, HB)
                nc.vector.tensor_mul(xq[0:8, s], xq[0:8, s], xq[0:8, s])

            # per half h: lw1/lw2 matmuls -> exp -> ht/hd matmuls -> p muls
            # -> ones-reduce into nd[h] -> scalar copy -> DMA out
            for h in range(2):
                s = bass.ts(h, HB)
                nc.tensor.matmul(lw[2 * h][:, :], lhsT=wb[:, 0:T], rhs=xq[:, s])
                nc.tensor.matmul(lw[2 * h + 1][:, :], lhsT=wb[:, T : 2 * T], rhs=xq[:, s])
                nc.scalar.activation(w[:, bass.ts(h, HB)], lw[2 * h][:, :], AF.Exp)
                nc.scalar.activation(
                    w[:, bass.ds(BC + h * HB, HB)], lw[2 * h + 1][:, :], AF.Exp
                )
                nc.tensor.matmul(ht[h][:, :], lhsT=c2, rhs=w[:, s])
                nc.tensor.matmul(hd[h][:, :], lhsT=d2, rhs=w[:, s])
                w2h = w[:, bass.ds(BC + h * HB, HB)]
                nc.vector.tensor_mul(p[:, bass.ts(2 * h, HB)], ht[h][:, :], w2h)
                nc.vector.tensor_mul(p[:, bass.ts(2 * h + 1, HB)], hd[h][:, :], w2h)
                nc.tensor.matmul(
                    nd[h][0:32, :], lhsT=ones32, rhs=p[:, bass.ts(2 * h, HB)]
                )
                nc.tensor.matmul(
                    nd[h][32:64, :], lhsT=ones32, rhs=p[:, bass.ts(2 * h + 1, HB)]
                )
                nc.scalar.copy(outt[h][:, :], nd[h][0:33, :])

            nc.sync.dma_start(out=out_ext[0:2, :], in_=outt[0][0:33:32, :])
            nc.scalar.dma_start(out=out_ext[2:4, :], in_=outt[1][0:33:32, :])

    nc.compile()
    return nc


def _get_graph():
    if "nc" not in _CACHE:
        _CACHE["nc"] = _build_graph()
    return _CACHE["nc"]


def _prep_inputs(X, mu, sigma, consequents, rules):
    X = np.ascontiguousarray(np.asarray(X, dtype=np.float32))
    mu64 = np.asarray(mu, dtype=np.float64)
    c = np.asarray(consequents, dtype=np.float64).sum(axis=1)
    r = np.asarray(rules).astype(np.int64)

    a = -0.5 / (np.asarray(sigma, np.float64) ** 2)  # [F, M]

    # tuple digit j of t (digit 0 most significant), t in [0, 81)
    digits = (np.arange(T)[:, None] // np.array([27, 9, 3, 1])[None, :]) % 3  # [81, 4]

    # A[half]: rows 0:8 coeff for x^2 rows, 8:16 for x rows, 16:18 the
    # constant term split hi/lo (the matching xq rows are 1.0)
    wb = np.zeros((K, 2 * T), np.float16)
    for half in range(2):
        A = np.zeros((16, T), np.float64)
        b = np.full(T, SHIFT, np.float64)
        for j in range(4):
            f = 4 * half + j
            d = digits[:, j]
            A[f, :] = a[f, d]
            A[8 + f, :] = -2.0 * a[f, d] * mu64[f, d]
            b += a[f, d] * mu64[f, d] ** 2
        wb[0:16, half * T : (half + 1) * T] = A.astype(np.float16)
        b_hi = b.astype(np.float16)
        b_lo = (b - b_hi.astype(np.float64)).astype(np.float16)
        wb[16, half * T : (half + 1) * T] = b_hi
        wb[17, half * T : (half + 1) * T] = b_lo

    rho1 = ((r[:, 0] * 3 + r[:, 1]) * 3 + r[:, 2]) * 3 + r[:, 3]
    rho2 = ((r[:, 4] * 3 + r[:, 5]) * 3 + r[:, 6]) * 3 + r[:, 7]
    C2 = np.zeros((T, T), np.float64)
    np.add.at(C2, (rho1, rho2), c)
    D2 = np.zeros((T, T), np.float64)
    np.add.at(D2, (rho1, rho2), 1.0)

    bigc = np.zeros((T, 2 * T + 32), np.float16)
    bigc[:, 0:T] = C2.astype(np.float16)
    bigc[:, T : 2 * T] = D2.astype(np.float16)
    bigc[:, 2 * T :] = 1.0
    bigc = np.ascontiguousarray(bigc)

    Xsh = X.reshape(NC, BC, F)
    xq = np.empty((NC, K, BC), np.float16)
    xt = np.swapaxes(Xsh, 1, 2).astype(np.float16)  # [NC, F, BC]
    xq[:, 0:8, :] = xt  # squared in place on device
    xq[:, 8:16, :] = xt
    xq[:, 16:18, :] = 1.0

    in_maps = [
        {"xq": np.ascontiguousarray(xq[i]), "wb": wb, "bigc": bigc}
        for i in range(NC)
    ]
    return in_maps


def _run(in_maps, trace=False, **kwargs):
    nc = _get_graph()
    return run_bass_kernel_spmd(
        nc, in_maps, core_ids=list(range(NC)), trace=trace, **kwargs
    )


def kernel(X, mu, sigma, consequents, rules):
    in_maps = _prep_inputs(X, mu, sigma, consequents, rules)
    res = _run(in_maps)
    eps = np.float32(1e-10 * np.exp(2.0 * SHIFT))
    outs = []
    for i in range(NC):
        o = np.asarray(res.results[i]["o"], dtype=np.float32)  # [4, HB]
        outs.append(o[0] / (o[1] + eps))
        outs.append(o[2] / (o[3] + eps))
    return np.concatenate(outs).astype(np.float32)


# revision 10
# speedup vs baseline: 1.0800x; 1.0072x over previous
"""Trainium2 Bass kernel for the ANFIS forward pass (8-core data-parallel).

Math: with L[b,f,m] = -0.5*((X[b,f]-mu[f,m])/sigma[f,m])^2,
  miAlloc[b,r] = prod_f exp(L[b,f,rules[r,f]])
  out[b] = (miAlloc @ c) / (sum_r miAlloc + 1e-10),  c = consequents.sum(1)

Factor the 8 features into two halves of 4. Each half has 81 possible
membership tuples, so miAlloc[b,r] = W1[b,rho1(r)] * W2[b,rho2(r)] where
  W1[b,t] = exp(sum_{f<4} a[f,tf]*(X[b,f]-mu[f,tf])^2),  a = -0.5/sigma^2
and rho1/rho2 map each rule to its half-tuple index. With
  C2[t1,t2] = sum_{r: rho(r)=(t1,t2)} c[r],   D2[t1,t2] = #{r: rho(r)=(t1,t2)}
(exact for arbitrary `rules`, duplicates included):
  num[b] = sum_{t2} (C2^T W1)[t2,b] * W2[t2,b]
  den[b] = sum_{t2} (D2^T W1)[t2,b] * W2[t2,b]
  out[b] = num[b] / (den[b] + 1e-10)     <- divide happens on HOST

Device-side design notes (all fp16 data path, fp32 PSUM accumulation):
 * logW is computed as a single K=18 matmul over z = [x(8) | x^2(8) | 1 | 1]:
   a*(x-mu)^2 = a*x^2 - 2*a*mu*x + a*mu^2. The quadratic x^2 rows are squared
   in place on VectorE (2-byte 2x mode); the two ones-rows carry the constant
   term split hi/lo across two fp16 rows so it lands with ~fp32 precision.
 * exp() values are scaled by e^SHIFT per half to stay out of fp16 subnormals;
   the scale cancels in num/den (host divides with a rescaled epsilon).
 * The PE p-state ramps 0.65->1.2->2.4 GHz with ~3us of continuous work, so
   a run of warm-up matmuls on garbage SBUF keeps the array busy through the
   framework preamble + input DMA flight; real matmuls then run at full rate.
 * num/den are reduced by ones-matmuls into one [64,512] PSUM tile per half,
   copied to SBUF on ScalarE (DMA cannot read PSUM), and DMA'd out as two
   rows; the final divide is elementwise host post-processing of the gather.
"""

import numpy as np

import concourse.bass as bass
import concourse.tile as tile
from concourse import bacc, mybir
from concourse.bass_utils import run_bass_kernel_spmd

B, F, M = 8192, 8, 3
NC = 8
BC = B // NC  # 1024 batch rows per core
HB = BC // 2  # 512-column half
T = M**4  # 81 tuples per feature-half
K = 18  # x(8) | x^2(8) | ones(2)
FP32 = mybir.dt.float32
FP16 = mybir.dt.float16
AF = mybir.ActivationFunctionType
SHIFT = 2.0  # per-half exp scale; cancels in num/den
N_WARM = 30  # PE p-state warm-up matmuls

_CACHE = {}


def _build_graph():
    nc = bacc.Bacc("TRN2", target_bir_lowering=False, debug=False, num_devices=NC)

    xq_ext = nc.dram_tensor("xq", [K, BC], FP16, kind="ExternalInput").ap()
    # wb: A1 | A2 stage-1 weights, [18, 162]
    wb_ext = nc.dram_tensor("wb", [K, 2 * T], FP16, kind="ExternalInput").ap()
    # bigc: C2 | D2 | ones32, [81, 194]
    bigc_ext = nc.dram_tensor("bigc", [T, 2 * T + 32], FP16, kind="ExternalInput").ap()
    # o rows: num_h0, den_h0, num_h1, den_h1
    out_ext = nc.dram_tensor("o", [4, HB], FP32, kind="ExternalOutput").ap()

    with tile.TileContext(nc) as tc:
        with (
            tc.tile_pool(name="const", bufs=1) as const,
            tc.tile_pool(name="work", bufs=1) as work,
            tc.tile_pool(name="psum", bufs=1, space=bass.MemorySpace.PSUM) as psum,
        ):
            xq = const.tile([K, BC], FP16)
            wb = const.tile([K, 2 * T], FP16)
            bigc = const.tile([T, 2 * T + 32], FP16)
            c2 = bigc[:, 0:T]
            d2 = bigc[:, T : 2 * T]
            ones32 = bigc[:, 2 * T : 2 * T + 32]

            # input DMAs: batch halves on the sync HWDGE queue (h0 gates
            # everything), stage-1 weights on the scalar queue (lands well
            # before the first matmul needs them), C2/D2/ones on the gpsimd
            # SWDGE queue (needed ~2.5us after trigger, latency hidden)
            nc.sync.dma_start(out=xq[:, 0:HB], in_=xq_ext[:, 0:HB])
            nc.sync.dma_start(out=xq[:, HB:BC], in_=xq_ext[:, HB:BC])
            nc.scalar.dma_start(out=wb[:, :], in_=wb_ext[:, :])
            nc.gpsimd.dma_start(out=bigc[:, :], in_=bigc_ext[:, :])

            # PSUM: 8 banks, tags reused once the lw tiles are consumed
            warm = psum.tile([T, HB], FP32, tag="pc", name="warm")
            lw = [
                psum.tile([T, HB], FP32, tag=t, name=f"lw{t}")
                for t in ("pa", "pb", "pc", "pd")
            ]  # w1h0, w2h0, w1h1, w2h1
            ht = [psum.tile([T, HB], FP32, tag=t, name=f"ht{t}") for t in ("pe", "pf")]
            hd = [psum.tile([T, HB], FP32, tag=t, name=f"hd{t}") for t in ("pg", "ph")]
            nd = [
                psum.tile([64, HB], FP32, tag=t, name=f"nd{t}") for t in ("pa", "pb")
            ]  # rows 0:32 num (row 0 used), 32:64 den (row 32 used)

            w = work.tile([T, 2 * BC], FP16)  # w1 cols 0:BC, w2 cols BC:2BC
            p = work.tile([T, 2 * BC], FP16)  # p1h0 | pdh0 | p1h1 | pdh1
            outt = [work.tile([33, HB], FP32, name=f"outt{h}") for h in range(2)]
            warm_l = work.tile([K, T], FP16)

            # PE warm-up: gated only on a tiny vector memset, so it runs from
            # the branch into the kernel body, ramping the p-state while the
            # input DMAs are in flight
            nc.vector.memset(warm_l[:, :], 0.0)
            for _ in range(N_WARM):
                nc.tensor.matmul(warm[:, 0:T], lhsT=warm_l[:, :], rhs=warm_l[:, :])

            # x^2 rows 0:8 squared in place (fp16 all-SBUF -> DVE 2x mode;
            # rows start at partition 0 to satisfy DVE partition alignment)
            for h in range(2):
                s = bass.ts(h, HB)
                nc.vector.tensor_mul(xq[0:8, s], xq[0:8, s], xq[0:8, s])

            # per half h: lw1/lw2 matmuls -> exp -> ht/hd matmuls -> p muls
            # -> ones-reduce into nd[h] -> scalar copy -> DMA out
            for h in range(2):
                s = bass.ts(h, HB)
                nc.tensor.matmul(lw[2 * h][:, :], lhsT=wb[:, 0:T], rhs=xq[:, s])
                nc.tensor.matmul(lw[2 * h + 1][:, :], lhsT=wb[:, T : 2 * T], rhs=xq[:, s])
                nc.scalar.activation(w[:, bass.ts(h, HB)], lw[2 * h][:, :], AF.Exp)
                nc.scalar.activation(
                    w[:, bass.ds(BC + h * HB, HB)], lw[2 * h + 1][:, :], AF.Exp
                )
                nc.tensor.matmul(ht[h][:, :], lhsT=c2, rhs=w[:, s])
                nc.tensor.matmul(hd[h][:, :], lhsT=d2, rhs=w[:, s])
                w2h = w[:, bass.ds(BC + h * HB, HB)]
                nc.vector.tensor_mul(p[:, bass.ts(2 * h, HB)], ht[h][:, :], w2h)
                nc.vector.tensor_mul(p[:, bass.ts(2 * h + 1, HB)], hd[h][:, :], w2h)
                nc.tensor.matmul(
                    nd[h][0:32, :], lhsT=ones32, rhs=p[:, bass.ts(2 * h, HB)]
                )
                nc.tensor.matmul(
                    nd[h][32:64, :], lhsT=ones32, rhs=p[:, bass.ts(2 * h + 1, HB)]
                )
                nc.scalar.copy(outt[h][:, :], nd[h][0:33, :])

            nc.sync.dma_start(out=out_ext[0:2, :], in_=outt[0][0:33:32, :])
            nc.sync.dma_start(out=out_ext[2:4, :], in_=outt[1][0:33:32, :])

    nc.compile()
    return nc


def _get_graph():
    if "nc" not in _CACHE:
        _CACHE["nc"] = _build_graph()
    return _CACHE["nc"]


def _prep_inputs(X, mu, sigma, consequents, rules):
    X = np.ascontiguousarray(np.asarray(X, dtype=np.float32))
    mu64 = np.asarray(mu, dtype=np.float64)
    c = np.asarray(consequents, dtype=np.float64).sum(axis=1)
    r = np.asarray(rules).astype(np.int64)

    a = -0.5 / (np.asarray(sigma, np.float64) ** 2)  # [F, M]

    # tuple digit j of t (digit 0 most significant), t in [0, 81)
    digits = (np.arange(T)[:, None] // np.array([27, 9, 3, 1])[None, :]) % 3  # [81, 4]

    # A[half]: rows 0:8 coeff for x^2 rows, 8:16 for x rows, 16:18 the
    # constant term split hi/lo (the matching xq rows are 1.0)
    wb = np.zeros((K, 2 * T), np.float16)
    for half in range(2):
        A = np.zeros((16, T), np.float64)
        b = np.full(T, SHIFT, np.float64)
        for j in range(4):
            f = 4 * half + j
            d = digits[:, j]
            A[f, :] = a[f, d]
            A[8 + f, :] = -2.0 * a[f, d] * mu64[f, d]
            b += a[f, d] * mu64[f, d] ** 2
        wb[0:16, half * T : (half + 1) * T] = A.astype(np.float16)
        b_hi = b.astype(np.float16)
        b_lo = (b - b_hi.astype(np.float64)).astype(np.float16)
        wb[16, half * T : (half + 1) * T] = b_hi
        wb[17, half * T : (half + 1) * T] = b_lo

    rho1 = ((r[:, 0] * 3 + r[:, 1]) * 3 + r[:, 2]) * 3 + r[:, 3]
    rho2 = ((r[:, 4] * 3 + r[:, 5]) * 3 + r[:, 6]) * 3 + r[:, 7]
    C2 = np.zeros((T, T), np.float64)
    np.add.at(C2, (rho1, rho2), c)
    D2 = np.zeros((T, T), np.float64)
    np.add.at(D2, (rho1, rho2), 1.0)

    bigc = np.zeros((T, 2 * T + 32), np.float16)
    bigc[:, 0:T] = C2.astype(np.float16)
    bigc[:, T : 2 * T] = D2.astype(np.float16)
    bigc[:, 2 * T :] = 1.0
    bigc = np.ascontiguousarray(bigc)

    Xsh = X.reshape(NC, BC, F)
    xq = np.empty((NC, K, BC), np.float16)
    xt = np.swapaxes(Xsh, 1, 2).astype(np.float16)  # [NC, F, BC]
    xq[:, 0:8, :] = xt  # squared in place on device
    xq[:, 8:16, :] = xt
    xq[:, 16:18, :] = 1.0

    in_maps = [
        {"xq": np.ascontiguousarray(xq[i]), "wb": wb, "bigc": bigc}
        for i in range(NC)
    ]
    return in_maps


def _run(in_maps, trace=False, **kwargs):
    nc = _get_graph()
    return run_bass_kernel_spmd(
        nc, in_maps, core_ids=list(range(NC)), trace=trace, **kwargs
    )


def kernel(X, mu, sigma, consequents, rules):
    in_maps = _prep_inputs(X, mu, sigma, consequents, rules)
    res = _run(in_maps)
    eps = np.float32(1e-10 * np.exp(2.0 * SHIFT))
    outs = []
    for i in range(NC):
        o = np.asarray(res.results[i]["o"], dtype=np.float32)  # [4, HB]
        outs.append(o[0] / (o[1] + eps))
        outs.append(o[2] / (o[3] + eps))
    return np.concatenate(outs).astype(np.float32)


# revision 14
# speedup vs baseline: 1.1246x; 1.0413x over previous
"""Trainium2 Bass kernel for the ANFIS forward pass (8-core data-parallel).

Math: with L[b,f,m] = -0.5*((X[b,f]-mu[f,m])/sigma[f,m])^2,
  miAlloc[b,r] = prod_f exp(L[b,f,rules[r,f]])
  out[b] = (miAlloc @ c) / (sum_r miAlloc + 1e-10),  c = consequents.sum(1)

Factor the 8 features into two halves of 4. Each half has 81 possible
membership tuples, so miAlloc[b,r] = W1[b,rho1(r)] * W2[b,rho2(r)] where
  W1[b,t] = exp(sum_{f<4} a[f,tf]*(X[b,f]-mu[f,tf])^2),  a = -0.5/sigma^2
and rho1/rho2 map each rule to its half-tuple index. With
  C2[t1,t2] = sum_{r: rho(r)=(t1,t2)} c[r],   D2[t1,t2] = #{r: rho(r)=(t1,t2)}
(exact for arbitrary `rules`, duplicates included):
  num[b] = sum_{t2} (C2^T W1)[t2,b] * W2[t2,b]
  den[b] = sum_{t2} (D2^T W1)[t2,b] * W2[t2,b]
  out[b] = num[b] / (den[b] + 1e-10)     <- divide happens on HOST

Device-side design notes (all fp16 data path, fp32 PSUM accumulation):
 * logW is computed as a single K=18 matmul over z = [x(8) | x^2(8) | 1 | 1]:
   a*(x-mu)^2 = a*x^2 - 2*a*mu*x + a*mu^2. The quadratic x^2 rows are squared
   in place on VectorE (2-byte 2x mode); the two ones-rows carry the constant
   term split hi/lo across two fp16 rows so it lands with ~fp32 precision.
 * exp() values are scaled by e^SHIFT per half to stay out of fp16 subnormals;
   the scale cancels in num/den (host divides with a rescaled epsilon).
 * The PE p-state ramps 0.65->1.2->2.4 GHz with ~3us of continuous work, so
   a run of warm-up matmuls on garbage SBUF keeps the array busy through the
   framework preamble + input DMA flight; real matmuls then run at full rate.
 * num/den are reduced by ones-matmuls into one [64,512] PSUM tile per half,
   copied to SBUF on ScalarE (DMA cannot read PSUM), and DMA'd out as two
   rows; the final divide is elementwise host post-processing of the gather.
"""

import numpy as np

import concourse.bass as bass
import concourse.tile as tile
from concourse import bacc, mybir
from concourse.bass_utils import run_bass_kernel_spmd

B, F, M = 8192, 8, 3
NC = 8
BC = B // NC  # 1024 batch rows per core
HB = BC // 2  # 512-column half
T = M**4  # 81 tuples per feature-half
K = 18  # x(8) | x^2(8) | ones(2)
FP32 = mybir.dt.float32
FP16 = mybir.dt.float16
AF = mybir.ActivationFunctionType
SHIFT = 2.0  # per-half exp scale; cancels in num/den
N_WARM = 6  # PE p-state warm-up matmuls (cold->mid ramp; HW won't boost past 1.2GHz)

_CACHE = {}


def _build_graph(sep):
    """sep=True: D2 is rank-1 (u v^T / s), den computed as (u^T W1)(v^T W2)/s
    on the host from shipped s1/s2 rows. sep=False: general D2 path with the
    hd matmuls + pd muls + den reduce on device."""
    nc = bacc.Bacc("TRN2", target_bir_lowering=False, debug=False, num_devices=NC)

    xq_ext = nc.dram_tensor("xq", [K, BC], FP16, kind="ExternalInput").ap()
    # wb: A1 | A2 stage-1 weights, [18, 162]
    wb_ext = nc.dram_tensor("wb", [K, 2 * T], FP16, kind="ExternalInput").ap()
    # bigc: sep: C2 | ones,u,v [81, 84]; general: C2 | D2 | ones [81, 163]
    BW = 2 * T + 1 if not sep else T + 3
    bigc_ext = nc.dram_tensor("bigc", [T, BW], FP16, kind="ExternalInput").ap()
    # o rows per half: sep: num, s1, s2 ([6, HB]); general: num, den ([4, HB])
    out_ext = nc.dram_tensor("o", [6 if sep else 4, HB], FP32, kind="ExternalOutput").ap()

    with tile.TileContext(nc) as tc:
        with (
            tc.tile_pool(name="const", bufs=1) as const,
            tc.tile_pool(name="work", bufs=1) as work,
            tc.tile_pool(name="psum", bufs=1, space=bass.MemorySpace.PSUM) as psum,
        ):
            xq = const.tile([K, BC], FP16)
            wb = const.tile([K, 2 * T], FP16)
            bigc = const.tile([T, BW], FP16)
            c2 = bigc[:, 0:T]
            if sep:
                ones1 = bigc[:, T : T + 1]
                ucol = bigc[:, T + 1 : T + 2]
                vcol = bigc[:, T + 2 : T + 3]
            else:
                d2 = bigc[:, T : 2 * T]
                ones1 = bigc[:, 2 * T : 2 * T + 1]

            # input DMAs: batch halves on the sync HWDGE queue (h0 gates
            # everything), stage-1 weights on the scalar queue (lands well
            # before the first matmul needs them), C2 etc. on the gpsimd
            # SWDGE queue (needed ~2.5us after trigger, latency hidden)
            nc.sync.dma_start(out=xq[:, 0:HB], in_=xq_ext[:, 0:HB])
            nc.sync.dma_start(out=xq[:, HB:BC], in_=xq_ext[:, HB:BC])
            nc.scalar.dma_start(out=wb[:, :], in_=wb_ext[:, :])
            nc.gpsimd.dma_start(out=bigc[:, :], in_=bigc_ext[:, :])

            # PSUM: 8 banks, tags reused once the lw tiles are consumed
            warm = psum.tile([T, HB], FP32, tag="pc", name="warm")
            lw = [
                psum.tile([T, HB], FP32, tag=t, name=f"lw{t}")
                for t in ("pa", "pb", "pc", "pd")
            ]  # w1h0, w2h0, w1h1, w2h1
            ht = [psum.tile([T, HB], FP32, tag=t, name=f"ht{t}") for t in ("pe", "pf")]
            if not sep:
                hd = [
                    psum.tile([T, HB], FP32, tag=t, name=f"hd{t}")
                    for t in ("pg", "ph")
                ]
            # nd rows used: 0 = num, 32 = s1/den, 64 = s2 (sep only)
            ndrows = 96 if sep else 64
            nd = [
                psum.tile([ndrows, HB], FP32, tag=t, name=f"nd{t}")
                for t in ("pa", "pb")
            ]

            w = work.tile([T, 2 * BC], FP16)  # w1 cols 0:BC, w2 cols BC:2BC
            p = work.tile([T, 2 * BC], FP16)  # p1h0 | pdh0 | p1h1 | pdh1
            cprows = ndrows - 31
            outt = [work.tile([cprows, HB], FP32, name=f"outt{h}") for h in range(2)]
            warm_l = work.tile([K, T], FP16)

            # PE warm-up: gated only on a tiny vector memset, so it runs from
            # the branch into the kernel body, covering the cold->mid ramp
            nc.vector.memset(warm_l[:, :], 0.0)
            for _ in range(N_WARM):
                nc.tensor.matmul(warm[:, 0:T], lhsT=warm_l[:, :], rhs=warm_l[:, :])

            # x^2 rows 0:8 squared in place (fp16 all-SBUF -> DVE 2x mode;
            # rows start at partition 0 to satisfy DVE partition alignment)
            for h in range(2):
                s = bass.ts(h, HB)
                nc.vector.tensor_mul(xq[0:8, s], xq[0:8, s], xq[0:8, s])

            for h in range(2):
                s = bass.ts(h, HB)
                w1h = w[:, bass.ts(h, HB)]
                w2h = w[:, bass.ds(BC + h * HB, HB)]
                nc.tensor.matmul(lw[2 * h][:, :], lhsT=wb[:, 0:T], rhs=xq[:, s])
                nc.tensor.matmul(lw[2 * h + 1][:, :], lhsT=wb[:, T : 2 * T], rhs=xq[:, s])
                nc.scalar.activation(w1h, lw[2 * h][:, :], AF.Exp)
                nc.scalar.activation(w2h, lw[2 * h + 1][:, :], AF.Exp)
                nc.tensor.matmul(ht[h][:, :], lhsT=c2, rhs=w1h)
                if sep:
                    # den factors: s1 = u^T w1 (row 32), s2 = v^T w2 (row 64)
                    nc.tensor.matmul(nd[h][32:33, :], lhsT=ucol, rhs=w1h)
                    nc.tensor.matmul(nd[h][64:65, :], lhsT=vcol, rhs=w2h)
                else:
                    nc.tensor.matmul(hd[h][:, :], lhsT=d2, rhs=w1h)
                    nc.vector.tensor_mul(p[:, bass.ts(2 * h + 1, HB)], hd[h][:, :], w2h)
                    nc.tensor.matmul(
                        nd[h][32:33, :], lhsT=ones1, rhs=p[:, bass.ts(2 * h + 1, HB)]
                    )
                nc.vector.tensor_mul(p[:, bass.ts(2 * h, HB)], ht[h][:, :], w2h)
                nc.tensor.matmul(
                    nd[h][0:1, :], lhsT=ones1, rhs=p[:, bass.ts(2 * h, HB)]
                )
                nc.scalar.copy(outt[h][:, :], nd[h][0:cprows, :])

            nrow = 3 if sep else 2
            for h in range(2):
                nc.sync.dma_start(
                    out=out_ext[h * nrow : (h + 1) * nrow, :],
                    in_=outt[h][0 : cprows : 32, :],
                )

    nc.compile()
    return nc


def _get_graph(sep):
    key = f"nc{int(sep)}"
    if key not in _CACHE:
        _CACHE[key] = _build_graph(sep)
    return _CACHE[key]


def _prep_inputs(X, mu, sigma, consequents, rules):
    X = np.ascontiguousarray(np.asarray(X, dtype=np.float32))
    mu64 = np.asarray(mu, dtype=np.float64)
    c = np.asarray(consequents, dtype=np.float64).sum(axis=1)
    r = np.asarray(rules).astype(np.int64)

    a = -0.5 / (np.asarray(sigma, np.float64) ** 2)  # [F, M]

    # tuple digit j of t (digit 0 most significant), t in [0, 81)
    digits = (np.arange(T)[:, None] // np.array([27, 9, 3, 1])[None, :]) % 3  # [81, 4]

    # A[half]: rows 0:8 coeff for x^2 rows, 8:16 for x rows, 16:18 the
    # constant term split hi/lo (the matching xq rows are 1.0)
    wb = np.zeros((K, 2 * T), np.float16)
    for half in range(2):
        A = np.zeros((16, T), np.float64)
        b = np.full(T, SHIFT, np.float64)
        for j in range(4):
            f = 4 * half + j
            d = digits[:, j]
            A[f, :] = a[f, d]
            A[8 + f, :] = -2.0 * a[f, d] * mu64[f, d]
            b += a[f, d] * mu64[f, d] ** 2
        wb[0:16, half * T : (half + 1) * T] = A.astype(np.float16)
        b_hi = b.astype(np.float16)
        b_lo = (b - b_hi.astype(np.float64)).astype(np.float16)
        wb[16, half * T : (half + 1) * T] = b_hi
        wb[17, half * T : (half + 1) * T] = b_lo

    rho1 = ((r[:, 0] * 3 + r[:, 1]) * 3 + r[:, 2]) * 3 + r[:, 3]
    rho2 = ((r[:, 4] * 3 + r[:, 5]) * 3 + r[:, 6]) * 3 + r[:, 7]
    C2 = np.zeros((T, T), np.float64)
    np.add.at(C2, (rho1, rho2), c)
    D2 = np.zeros((T, T), np.float64)
    np.add.at(D2, (rho1, rho2), 1.0)

    # Separable den path when D2 is rank-1 with fp16-exact factors (true for
    # the reference's full cartesian-product rules: D2 is all-ones).
    u = D2.sum(axis=1)
    v = D2.sum(axis=0)
    s = D2.sum()
    sep = (
        s > 0
        and np.array_equal(np.outer(u, v) / s, D2 * 1.0)
        and np.array_equal(u.astype(np.float16).astype(np.float64), u)
        and np.array_equal(v.astype(np.float16).astype(np.float64), v)
    )
    _CACHE["sep"] = sep
    _CACHE["dscale"] = s

    if sep:
        bigc = np.zeros((T, T + 3), np.float16)
        bigc[:, 0:T] = C2.astype(np.float16)
        bigc[:, T] = 1.0
        bigc[:, T + 1] = u.astype(np.float16)
        bigc[:, T + 2] = v.astype(np.float16)
    else:
        bigc = np.zeros((T, 2 * T + 1), np.float16)
        bigc[:, 0:T] = C2.astype(np.float16)
        bigc[:, T : 2 * T] = D2.astype(np.float16)
        bigc[:, 2 * T] = 1.0
    bigc = np.ascontiguousarray(bigc)

    Xsh = X.reshape(NC, BC, F)
    xq = np.empty((NC, K, BC), np.float16)
    xt = np.swapaxes(Xsh, 1, 2).astype(np.float16)  # [NC, F, BC]
    xq[:, 0:8, :] = xt  # squared in place on device
    xq[:, 8:16, :] = xt
    xq[:, 16:18, :] = 1.0

    in_maps = [
        {"xq": np.ascontiguousarray(xq[i]), "wb": wb, "bigc": bigc}
        for i in range(NC)
    ]
    return in_maps


def _run(in_maps, trace=False, **kwargs):
    nc = _get_graph(_CACHE.get("sep", True))
    return run_bass_kernel_spmd(
        nc, in_maps, core_ids=list(range(NC)), trace=trace, **kwargs
    )


def kernel(X, mu, sigma, consequents, rules):
    in_maps = _prep_inputs(X, mu, sigma, consequents, rules)
    res = _run(in_maps)
    eps = np.float32(1e-10 * np.exp(2.0 * SHIFT))
    outs = []
    if _CACHE["sep"]:
        ds = np.float32(1.0 / _CACHE["dscale"])
        for i in range(NC):
            o = np.asarray(res.results[i]["o"], dtype=np.float32)  # [6, HB]
            outs.append(o[0] / (o[1] * o[2] * ds + eps))
            outs.append(o[3] / (o[4] * o[5] * ds + eps))
    else:
        for i in range(NC):
            o = np.asarray(res.results[i]["o"], dtype=np.float32)  # [4, HB]
            outs.append(o[0] / (o[1] + eps))
            outs.append(o[2] / (o[3] + eps))
    return np.concatenate(outs).astype(np.float32)


# revision 18
# speedup vs baseline: 1.1307x; 1.0055x over previous
"""Trainium2 Bass kernel for the ANFIS forward pass (8-core data-parallel).

Math: with L[b,f,m] = -0.5*((X[b,f]-mu[f,m])/sigma[f,m])^2,
  miAlloc[b,r] = prod_f exp(L[b,f,rules[r,f]])
  out[b] = (miAlloc @ c) / (sum_r miAlloc + 1e-10),  c = consequents.sum(1)

Factor the 8 features into two halves of 4. Each half has 81 possible
membership tuples, so miAlloc[b,r] = W1[b,rho1(r)] * W2[b,rho2(r)] where
  W1[b,t] = exp(sum_{f<4} a[f,tf]*(X[b,f]-mu[f,tf])^2),  a = -0.5/sigma^2
and rho1/rho2 map each rule to its half-tuple index. With
  C2[t1,t2] = sum_{r: rho(r)=(t1,t2)} c[r],   D2[t1,t2] = #{r: rho(r)=(t1,t2)}
(exact for arbitrary `rules`, duplicates included):
  num[b] = sum_{t2} (C2^T W1)[t2,b] * W2[t2,b]
  den[b] = sum_{t2} (D2^T W1)[t2,b] * W2[t2,b]
  out[b] = num[b] / (den[b] + 1e-10)     <- divide happens on HOST

Device-side design notes (all fp16 data path, fp32 PSUM accumulation):
 * logW is computed as a single K=18 matmul over z = [x(8) | x^2(8) | 1 | 1]:
   a*(x-mu)^2 = a*x^2 - 2*a*mu*x + a*mu^2. The quadratic x^2 rows are squared
   in place on VectorE (2-byte 2x mode); the two ones-rows carry the constant
   term split hi/lo across two fp16 rows so it lands with ~fp32 precision.
 * exp() values are scaled by e^SHIFT per half to stay out of fp16 subnormals;
   the scale cancels in num/den (host divides with a rescaled epsilon).
 * The PE p-state ramps 0.65->1.2->2.4 GHz with ~3us of continuous work, so
   a run of warm-up matmuls on garbage SBUF keeps the array busy through the
   framework preamble + input DMA flight; real matmuls then run at full rate.
 * num/den are reduced by ones-matmuls into one [64,512] PSUM tile per half,
   copied to SBUF on ScalarE (DMA cannot read PSUM), and DMA'd out as two
   rows; the final divide is elementwise host post-processing of the gather.
"""

import numpy as np

import concourse.bass as bass
import concourse.tile as tile
from concourse import bacc, mybir
from concourse.bass_utils import run_bass_kernel_spmd

B, F, M = 8192, 8, 3
NC = 8
BC = B // NC  # 1024 batch rows per core
HB = BC // 2  # 512-column half
T = M**4  # 81 tuples per feature-half
K = 18  # x(8) | x^2(8) | ones(2)
FP32 = mybir.dt.float32
FP16 = mybir.dt.float16
AF = mybir.ActivationFunctionType
SHIFT = 2.0  # per-half exp scale; cancels in num/den
N_WARM = 6  # PE p-state warm-up matmuls (cold->mid ramp; HW won't boost past 1.2GHz)

_CACHE = {}


def _build_graph(sep):
    """sep=True: D2 is rank-1 (u v^T / s), den computed as (u^T W1)(v^T W2)/s
    on the host from shipped s1/s2 rows. sep=False: general D2 path with the
    hd matmuls + pd muls + den reduce on device."""
    nc = bacc.Bacc("TRN2", target_bir_lowering=False, debug=False, num_devices=NC)

    # xqw: batch-half h0 | stage-1 weights A1,A2 | batch-half h1. Embedding
    # the weights in the h0 transfer delivers them with the first DMA instead
    # of gating the first matmul on a separate (late) weights DMA.
    XW = BC + 2 * T  # 1186 columns
    xqw_ext = nc.dram_tensor("xqw", [K, XW], FP16, kind="ExternalInput").ap()
    # bigc: sep: C2 | ones,u,v [81, 84]; general: C2 | D2 | ones [81, 163]
    BW = 2 * T + 1 if not sep else T + 3
    bigc_ext = nc.dram_tensor("bigc", [T, BW], FP16, kind="ExternalInput").ap()
    # o rows per half: sep: num, s1, s2 ([6, HB]); general: num, den ([4, HB])
    out_ext = nc.dram_tensor("o", [6 if sep else 4, HB], FP32, kind="ExternalOutput").ap()

    with tile.TileContext(nc) as tc:
        with (
            tc.tile_pool(name="const", bufs=1) as const,
            tc.tile_pool(name="work", bufs=1) as work,
            tc.tile_pool(name="psum", bufs=1, space=bass.MemorySpace.PSUM) as psum,
        ):
            xqw = const.tile([K, XW], FP16)
            xh = [xqw[:, 0:HB], xqw[:, HB + 2 * T : XW]]
            wb = xqw[:, HB : HB + 2 * T]
            bigc = const.tile([T, BW], FP16)
            c2 = bigc[:, 0:T]
            if sep:
                ones1 = bigc[:, T : T + 1]
                ucol = bigc[:, T + 1 : T + 2]
                vcol = bigc[:, T + 2 : T + 3]
            else:
                d2 = bigc[:, T : 2 * T]
                ones1 = bigc[:, 2 * T : 2 * T + 1]

            # input DMAs on the sync HWDGE queue: h0+weights first (gates
            # everything), then h1; C2 etc. on the gpsimd SWDGE queue
            # (needed ~2.5us after trigger, latency hidden)
            nc.sync.dma_start(out=xqw[:, 0 : HB + 2 * T], in_=xqw_ext[:, 0 : HB + 2 * T])
            nc.sync.dma_start(
                out=xqw[:, HB + 2 * T : XW], in_=xqw_ext[:, HB + 2 * T : XW]
            )
            nc.gpsimd.dma_start(out=bigc[:, :], in_=bigc_ext[:, :])

            # PSUM: 8 banks, tags reused once the lw tiles are consumed
            warm = psum.tile([T, HB], FP32, tag="pc", name="warm")
            lw = [
                psum.tile([T, HB], FP32, tag=t, name=f"lw{t}")
                for t in ("pa", "pb", "pc", "pd")
            ]  # w1h0, w2h0, w1h1, w2h1
            ht = [psum.tile([T, HB], FP32, tag=t, name=f"ht{t}") for t in ("pe", "pf")]
            if not sep:
                hd = [
                    psum.tile([T, HB], FP32, tag=t, name=f"hd{t}")
                    for t in ("pg", "ph")
                ]
            # nd rows used: 0 = num, 32 = s1/den, 64 = s2 (sep only)
            ndrows = 96 if sep else 64
            nd = [
                psum.tile([ndrows, HB], FP32, tag=t, name=f"nd{t}")
                for t in ("pa", "pb")
            ]

            w = work.tile([T, 2 * BC], FP16)  # w1 cols 0:BC, w2 cols BC:2BC
            p = work.tile([T, 2 * BC], FP16)  # p1h0 | pdh0 | p1h1 | pdh1
            cprows = ndrows - 31
            outt = [work.tile([cprows, HB], FP32, name=f"outt{h}") for h in range(2)]
            warm_l = work.tile([K, T], FP16)

            # PE warm-up: gated only on a tiny vector memset, so it runs from
            # the branch into the kernel body, covering the cold->mid ramp
            nc.vector.memset(warm_l[:, :], 0.0)
            for _ in range(N_WARM):
                nc.tensor.matmul(warm[:, 0:T], lhsT=warm_l[:, :], rhs=warm_l[:, :])

            # x^2 rows 0:8 squared in place (fp16 all-SBUF -> DVE 2x mode;
            # rows start at partition 0 to satisfy DVE partition alignment)
            for h in range(2):
                nc.vector.tensor_mul(xh[h][0:8, :], xh[h][0:8, :], xh[h][0:8, :])

            for h in range(2):
                w1h = w[:, bass.ts(h, HB)]
                w2h = w[:, bass.ds(BC + h * HB, HB)]
                nc.tensor.matmul(lw[2 * h][:, :], lhsT=wb[:, 0:T], rhs=xh[h])
                nc.tensor.matmul(lw[2 * h + 1][:, :], lhsT=wb[:, T : 2 * T], rhs=xh[h])
                nc.scalar.activation(w1h, lw[2 * h][:, :], AF.Exp)
                nc.scalar.activation(w2h, lw[2 * h + 1][:, :], AF.Exp)
                nc.tensor.matmul(ht[h][:, :], lhsT=c2, rhs=w1h)
                if sep:
                    # den factors: s1 = u^T w1 (row 32), s2 = v^T w2 (row 64)
                    nc.tensor.matmul(nd[h][32:33, :], lhsT=ucol, rhs=w1h)
                    nc.tensor.matmul(nd[h][64:65, :], lhsT=vcol, rhs=w2h)
                else:
                    nc.tensor.matmul(hd[h][:, :], lhsT=d2, rhs=w1h)
                    nc.vector.tensor_mul(p[:, bass.ts(2 * h + 1, HB)], hd[h][:, :], w2h)
                    nc.tensor.matmul(
                        nd[h][32:33, :], lhsT=ones1, rhs=p[:, bass.ts(2 * h + 1, HB)]
                    )
                # h1 finale in 256-col chunks so Vector/PE/Scalar pipeline
                # into the output DMA instead of serializing full 512 ops
                nq = 2 if h == 1 else 1
                qw = HB // nq
                for q in range(nq):
                    cs = bass.ds(2 * h * HB + q * qw, qw)
                    ns = bass.ds(q * qw, qw)
                    nc.vector.tensor_mul(p[:, cs], ht[h][:, ns], w2h[:, ns])
                    nc.tensor.matmul(nd[h][0:1, ns], lhsT=ones1, rhs=p[:, cs])
                    nc.scalar.copy(outt[h][:, ns], nd[h][0:cprows, ns])

            nrow = 3 if sep else 2
            for h in range(2):
                nc.sync.dma_start(
                    out=out_ext[h * nrow : (h + 1) * nrow, :],
                    in_=outt[h][0 : cprows : 32, :],
                )

    nc.compile()
    return nc


def _get_graph(sep):
    key = f"nc{int(sep)}"
    if key not in _CACHE:
        _CACHE[key] = _build_graph(sep)
    return _CACHE[key]


def _prep_inputs(X, mu, sigma, consequents, rules):
    X = np.ascontiguousarray(np.asarray(X, dtype=np.float32))
    mu64 = np.asarray(mu, dtype=np.float64)
    c = np.asarray(consequents, dtype=np.float64).sum(axis=1)
    r = np.asarray(rules).astype(np.int64)

    a = -0.5 / (np.asarray(sigma, np.float64) ** 2)  # [F, M]

    # tuple digit j of t (digit 0 most significant), t in [0, 81)
    digits = (np.arange(T)[:, None] // np.array([27, 9, 3, 1])[None, :]) % 3  # [81, 4]

    # A[half]: rows 0:8 coeff for x^2 rows, 8:16 for x rows, 16:18 the
    # constant term split hi/lo (the matching xq rows are 1.0)
    wb = np.zeros((K, 2 * T), np.float16)
    for half in range(2):
        A = np.zeros((16, T), np.float64)
        b = np.full(T, SHIFT, np.float64)
        for j in range(4):
            f = 4 * half + j
            d = digits[:, j]
            A[f, :] = a[f, d]
            A[8 + f, :] = -2.0 * a[f, d] * mu64[f, d]
            b += a[f, d] * mu64[f, d] ** 2
        wb[0:16, half * T : (half + 1) * T] = A.astype(np.float16)
        b_hi = b.astype(np.float16)
        b_lo = (b - b_hi.astype(np.float64)).astype(np.float16)
        wb[16, half * T : (half + 1) * T] = b_hi
        wb[17, half * T : (half + 1) * T] = b_lo

    rho1 = ((r[:, 0] * 3 + r[:, 1]) * 3 + r[:, 2]) * 3 + r[:, 3]
    rho2 = ((r[:, 4] * 3 + r[:, 5]) * 3 + r[:, 6]) * 3 + r[:, 7]
    C2 = np.zeros((T, T), np.float64)
    np.add.at(C2, (rho1, rho2), c)
    D2 = np.zeros((T, T), np.float64)
    np.add.at(D2, (rho1, rho2), 1.0)

    # Separable den path when D2 is rank-1 with fp16-exact factors (true for
    # the reference's full cartesian-product rules: D2 is all-ones).
    u = D2.sum(axis=1)
    v = D2.sum(axis=0)
    s = D2.sum()
    sep = (
        s > 0
        and np.array_equal(np.outer(u, v) / s, D2 * 1.0)
        and np.array_equal(u.astype(np.float16).astype(np.float64), u)
        and np.array_equal(v.astype(np.float16).astype(np.float64), v)
    )
    _CACHE["sep"] = sep
    _CACHE["dscale"] = s

    if sep:
        bigc = np.zeros((T, T + 3), np.float16)
        bigc[:, 0:T] = C2.astype(np.float16)
        bigc[:, T] = 1.0
        bigc[:, T + 1] = u.astype(np.float16)
        bigc[:, T + 2] = v.astype(np.float16)
    else:
        bigc = np.zeros((T, 2 * T + 1), np.float16)
        bigc[:, 0:T] = C2.astype(np.float16)
        bigc[:, T : 2 * T] = D2.astype(np.float16)
        bigc[:, 2 * T] = 1.0
    bigc = np.ascontiguousarray(bigc)

    Xsh = X.reshape(NC, BC, F)
    xqw = np.empty((NC, K, BC + 2 * T), np.float16)  # xh0 | A1,A2 | xh1
    xt = np.swapaxes(Xsh, 1, 2).astype(np.float16)  # [NC, F, BC]
    for blk, s in ((slice(0, HB), slice(0, HB)), (slice(HB + 2 * T, None), slice(HB, BC))):
        xqw[:, 0:8, blk] = xt[:, :, s]  # squared in place on device
        xqw[:, 8:16, blk] = xt[:, :, s]
        xqw[:, 16:18, blk] = 1.0
    xqw[:, :, HB : HB + 2 * T] = wb[None, :, :]

    in_maps = [{"xqw": np.ascontiguousarray(xqw[i]), "bigc": bigc} for i in range(NC)]
    return in_maps


def _run(in_maps, trace=False, **kwargs):
    nc = _get_graph(_CACHE.get("sep", True))
    return run_bass_kernel_spmd(
        nc, in_maps, core_ids=list(range(NC)), trace=trace, **kwargs
    )


def kernel(X, mu, sigma, consequents, rules):
    in_maps = _prep_inputs(X, mu, sigma, consequents, rules)
    res = _run(in_maps)
    eps = np.float32(1e-10 * np.exp(2.0 * SHIFT))
    outs = []
    if _CACHE["sep"]:
        ds = np.float32(1.0 / _CACHE["dscale"])
        for i in range(NC):
            o = np.asarray(res.results[i]["o"], dtype=np.float32)  # [6, HB]
            outs.append(o[0] / (o[1] * o[2] * ds + eps))
            outs.append(o[3] / (o[4] * o[5] * ds + eps))
    else:
        for i in range(NC):
            o = np.asarray(res.results[i]["o"], dtype=np.float32)  # [4, HB]
            outs.append(o[0] / (o[1] + eps))
            outs.append(o[2] / (o[3] + eps))
    return np.concatenate(outs).astype(np.float32)
